# revision 5
# baseline (speedup 1.0000x reference)
"""Entropy-gated multi-head attention on 8 Trainium2 NeuronCores.

Sharding: core c = b*4 + g handles batch b (of 2) and head-group g (4 of the
16 heads).  Tokens with gate==0 pass x through untouched and contribute
exactly zero k/v (zero biases), so the device only processes the compacted
active tokens (~half), with the softmax denominator corrected by the count of
inactive tokens: each inactive key contributes exp(0)=1 to the softmax sum
(scores vs. zeroed k are exactly 0) and nothing to the numerator (v=0).

Device math per core (no max-subtraction; scores are O(5) so exp is safe):
  QT = Wq_g^T x^T, KT = Wk_g^T x^T           [256, SA]
  V  = x Wv_g                                 [SA, 256]
  per head h: PT = exp((KT_h^T QT_h)/8)       [SA_k, SA_q]
              OT' = [V_h | 1]^T PT            [65, SA_q] (row 64 = colsum = Z_act)
              r = 1/(Z_act + (S - SA))        recip on [2,qn] THEN broadcast
              osb_h = OT * r                  (scaled attention out, lhsT form)
  Y(q, :) = sum_h osb_h^T Wo_h                (pair-packed K=128 matmuls)
Host sums the 4 per-group partial Y per batch, adds bo, scatters into x.

v2 changes vs baseline (183976ns):
  - Y matmuls pair-packed K=128 (half the Y instruction cycles)
  - reciprocal_approx_fast on [2,qn] pre-broadcast (was reciprocal on
    [64,qn] post-broadcast: ~30us of DVE)
  - CADD folded into the same small pre-broadcast op
  - y output in bf16 (half the output DMA)
  - s-granular software pipeline: scores(s) / exp(s) / AV(s-1) with filler
    jobs drained every round to keep the PE continuously busy (max p-state)
  - PSUM: st bufs=2 (1 bank each), ot bufs=4, aux bufs=2 -> exactly 8 banks
"""

import math
from contextlib import ExitStack

import numpy as np
import ml_dtypes

import concourse.bass as bass
import concourse.mybir as mybir
from concourse import bacc
import concourse.tile as tile
from concourse.bass_utils import run_bass_kernel_spmd

B, S, D = 2, 2048, 1024
H, DH = 16, 64
NCORES = 8
GROUPS = NCORES // B          # head-groups per batch = 4
HC = H // GROUPS              # heads per core = 4
DC = HC * DH                  # head-group width = 256

MM_DTYPE = "bf16"

_DT = {"bf16": mybir.dt.bfloat16, "f32": mybir.dt.float32}
_NPDT = {"bf16": ml_dtypes.bfloat16, "f32": np.float32}

f32 = mybir.dt.float32
bf16 = mybir.dt.bfloat16


def _chunks(total, step):
    out = []
    o = 0
    while o < total:
        out.append((o, min(step, total - o)))
        o += step
    return out


def _build(SA: int, dtype_tag: str) -> bass.Bass:
    DT = _DT[dtype_tag]
    nkt = D // 128            # 8 contraction tiles for projections
    nst = SA // 128           # token (key) tiles
    qch = _chunks(SA, 512)    # q chunks
    dch = _chunks(D, 512)     # output-dim chunks
    CADD = float(S - SA)      # inactive keys not represented by padding

    nc = bacc.Bacc()
    xT_d = nc.dram_tensor("xT", [D, SA], DT, kind="ExternalInput")
    wq_d = nc.dram_tensor("wq", [D, DC], DT, kind="ExternalInput")
    wk_d = nc.dram_tensor("wk", [D, DC], DT, kind="ExternalInput")
    wv_d = nc.dram_tensor("wv", [D, DC], DT, kind="ExternalInput")
    wo_d = nc.dram_tensor("wo", [DC, D], DT, kind="ExternalInput")
    y_d = nc.dram_tensor("y", [SA, D], bf16, kind="ExternalOutput")

    with tile.TileContext(nc) as tc, ExitStack() as ctx:
        singles = ctx.enter_context(tc.tile_pool(name="singles", bufs=1))
        pt_pool = ctx.enter_context(tc.tile_pool(name="pt", bufs=6))
        osb_pool = ctx.enter_context(tc.tile_pool(name="osb", bufs=3))
        otmp_pool = ctx.enter_context(tc.tile_pool(name="otmp", bufs=2))
        zsb_pool = ctx.enter_context(tc.tile_pool(name="zsb", bufs=2))
        rbc_pool = ctx.enter_context(tc.tile_pool(name="rbc", bufs=3))
        yout_pool = ctx.enter_context(tc.tile_pool(name="yout", bufs=3))
        zscr_pool = ctx.enter_context(tc.tile_pool(name="zscr", bufs=2,
                                                   space="DRAM"))
        # PSUM budget: st 2x1 + ot(pair) 2x2 + aux 2x1 = 8 banks
        st_ps = ctx.enter_context(tc.tile_pool(name="stps", bufs=2, space="PSUM"))
        ot_ps_pool = ctx.enter_context(tc.tile_pool(name="otps", bufs=2, space="PSUM"))
        aux_ps = ctx.enter_context(tc.tile_pool(name="auxps", bufs=2, space="PSUM"))

        # ---- persistent SBUF; DMA order = consumption order for fast start
        wk_sb = singles.tile([128, nkt, DC], DT)
        wq_sb = singles.tile([128, nkt, DC], DT)
        wv_sb = singles.tile([128, nkt, DC], DT)
        xt = singles.tile([128, nkt, SA], DT)
        (c0, c0n) = qch[0]
        for t in range(nkt):
            nc.sync.dma_start(wk_sb[:, t, :], wk_d[t * 128:(t + 1) * 128, :])
            nc.sync.dma_start(xt[:, t, c0:c0 + c0n],
                              xT_d[t * 128:(t + 1) * 128, c0:c0 + c0n])
            nc.sync.dma_start(wq_sb[:, t, :], wq_d[t * 128:(t + 1) * 128, :])
        for t in range(nkt):
            nc.sync.dma_start(wv_sb[:, t, :], wv_d[t * 128:(t + 1) * 128, :])
        for (q0, qn) in qch[1:]:
            for t in range(nkt):
                nc.sync.dma_start(xt[:, t, q0:q0 + qn],
                                  xT_d[t * 128:(t + 1) * 128, q0:q0 + qn])
        wo_sb = []
        for p in range(HC // 2):
            w = singles.tile([128, D], DT, tag=f"wo{p}", name=f"wo{p}")
            nc.sync.dma_start(w, wo_d[p * 128:(p + 1) * 128, :])
            wo_sb.append(w)

        # ---- projections: QT/KT [256, SA], V (augmented with ones) ----
        qt = [singles.tile([128, SA], DT, tag=f"qt{m}", name=f"qt{m}")
              for m in range(2)]
        kt = [singles.tile([128, SA], DT, tag=f"kt{m}", name=f"kt{m}")
              for m in range(2)]
        v_aug = singles.tile([128, nst, HC, 65], DT)

        # ACT exp-table preload: tiny dummy exp as the very first ACT
        # instruction so the ~1.3us table load overlaps the input DMAs.
        dummy = singles.tile([1, 16], f32, tag="dummy", name="dummy")
        nc.vector.memset(dummy, 0.0)
        nc.scalar.activation(dummy, dummy,
                             mybir.ActivationFunctionType.Exp, scale=1.0)
        nc.vector.memset(v_aug[:, :, :, 64:65], 1.0)

        def proj_qk(m, dst, w_sb, q0, qn):
            ps = aux_ps.tile([128, 512], f32, tag="aux", name="ps")
            for t in range(nkt):
                nc.tensor.matmul(
                    ps[:, :qn],
                    w_sb[:, t, m * 128:(m + 1) * 128],
                    xt[:, t, q0:q0 + qn],
                    start=(t == 0), stop=(t == nkt - 1))
            nc.vector.tensor_copy(dst[m][:, q0:q0 + qn], ps[:, :qn])

        v_done = set()

        def proj_v(s):
            v_done.add(s)
            ps = aux_ps.tile([128, 512], f32, tag="aux", name="ps")
            for t in range(nkt):
                nc.tensor.matmul(
                    ps[:, :DC],
                    xt[:, t, s * 128:(s + 1) * 128],
                    wv_sb[:, t, :],
                    start=(t == 0), stop=(t == nkt - 1))
            for h in range(HC):
                nc.vector.tensor_copy(v_aug[:, s, h, 0:64],
                                      ps[:, h * 64:(h + 1) * 64])

        # filler queue: independent PE work drained between pipeline rounds
        jobs = []

        def drain(k):
            for _ in range(min(k, len(jobs))):
                jobs.pop(0)()

        # warmup: only what chunk0/pair0 strictly needs, rest queued
        proj_qk(0, kt, wk_sb, c0, c0n)
        proj_qk(0, qt, wq_sb, c0, c0n)
        proj_v(0)
        proj_v(1)
        for s in range(2, nst):
            jobs.append(lambda s=s: proj_v(s))
        jobs.append(lambda: proj_qk(1, kt, wk_sb, c0, c0n))
        jobs.append(lambda: proj_qk(1, qt, wq_sb, c0, c0n))
        for (q0, qn) in qch[1:]:
            jobs.append(lambda a=q0, b=qn: proj_qk(0, kt, wk_sb, a, b))
            jobs.append(lambda a=q0, b=qn: proj_qk(0, qt, wq_sb, a, b))
            jobs.append(lambda a=q0, b=qn: proj_qk(1, kt, wk_sb, a, b))
            jobs.append(lambda a=q0, b=qn: proj_qk(1, qt, wq_sb, a, b))

        # output projection: pair-packed K=128 accumulating matmuls
        def y_job(q0, qn, jt, osb_pair):
            qtn = min(128, qn - jt * 128)
            for (d0, dn) in dch:
                yp = aux_ps.tile([128, 512], f32, tag="aux", name="yp")
                for p in range(HC // 2):
                    nc.tensor.matmul(
                        yp[:qtn, :dn],
                        osb_pair[p][:, jt * 128:jt * 128 + qtn],
                        wo_sb[p][:, d0:d0 + dn],
                        start=(p == 0), stop=(p == HC // 2 - 1))
                yo = yout_pool.tile([128, 512], bf16, tag="yo", name="yo")
                nc.vector.tensor_copy(yo[:qtn, :dn], yp[:qtn, :dn])
                nc.sync.dma_start(
                    y_d[q0 + jt * 128: q0 + jt * 128 + qtn, d0:d0 + dn],
                    yo[:qtn, :dn])

        # ---- attention: s-granular pipeline per (chunk, pair) ----
        for ci, (q0, qn) in enumerate(qch):
            osb_c = [None, None]
            for p in range(HC // 2):
                m = p
                h0, h1 = 2 * p, 2 * p + 1
                # both heads of the pair share one 2-bank psum tile so the
                # Z rows land side-by-side on partition 64 (one DVE op)
                ot_pair = ot_ps_pool.tile([65, 2, 512], f32, tag="ot",
                                          name="ot_ps")
                ot_ps = {h0: ot_pair[:, 0, :], h1: ot_pair[:, 1, :]}
                pt_prev = None
                for s in range(nst):
                    # filler first: keeps PE busy while ACT runs exp(s-1)
                    drain(2 if (ci == 0 and p == 0 and s < 6) else 1)
                    st = {}
                    for h in (h0, h1):
                        r0 = (h % 2) * 64
                        st[h] = st_ps.tile([128, 512], f32, tag="st",
                                           name="st_ps")
                        nc.tensor.matmul(
                            st[h][:, :qn],
                            kt[m][r0:r0 + 64, s * 128:(s + 1) * 128],
                            qt[m][r0:r0 + 64, q0:q0 + qn],
                            start=True, stop=True,
                            tile_position=(r0, 0))
                    pt = {}
                    for h in (h0, h1):
                        pt[h] = pt_pool.tile([128, 512], DT, tag="pt",
                                             name="pt")
                        nc.scalar.activation(
                            pt[h][:, :qn], st[h][:, :qn],
                            mybir.ActivationFunctionType.Exp, scale=0.125)
                    # AV lagged one round so exp(s-1) is done when PE gets here
                    if pt_prev is not None:
                        sp = s - 1
                        assert all(t in v_done for t in (sp,)), \
                            f"proj_v not emitted before OT consumer: {sp}"
                        for h in (h0, h1):
                            nc.tensor.matmul(
                                ot_ps[h][:, :qn],
                                v_aug[:, sp, h, :],
                                pt_prev[h][:, :qn],
                                start=(sp == 0), stop=False)
                    pt_prev = pt
                drain(1)
                sp = nst - 1
                assert sp in v_done
                for h in (h0, h1):
                    nc.tensor.matmul(
                        ot_ps[h][:, :qn],
                        v_aug[:, sp, h, :],
                        pt_prev[h][:, :qn],
                        start=(sp == 0), stop=True)

                # Z path: both Z rows live on partition 64 side by side;
                # +CADD and approx-reciprocal BEFORE the broadcast (the
                # baseline ran full reciprocal on the [64,qn] broadcast).
                zt = zsb_pool.tile([65, 2, 512], f32, tag="zt", name="zt")
                nc.vector.tensor_scalar(
                    out=zt[64:65, :, :qn], in0=ot_pair[64:65, :, :qn],
                    scalar1=CADD, scalar2=None, op0=mybir.AluOpType.add)
                nc.vector.reciprocal_approx_fast(zt[64:65, :, :qn],
                                                 zt[64:65, :, :qn])
                zd = zscr_pool.tile([1, 2, 512], f32, tag="zd", name="zd")
                nc.sync.dma_start(zd[:, :, :qn], zt[64:65, :, :qn])
                rb = rbc_pool.tile([64, 2, 512], f32, tag="rb", name="rb")
                for i in range(2):
                    nc.sync.dma_start(
                        rb[:, i, :qn],
                        zd[0:1, i, :qn].to_broadcast((64, qn)))
                osbp = osb_pool.tile([128, 512], DT, tag="osbp", name="osbp")
                osb_c[p] = osbp
                for i, h in enumerate((h0, h1)):
                    if i == 0:
                        nc.vector.scalar_tensor_tensor(
                            out=osbp[0:64, :qn], in0=ot_ps[h][0:64, :qn],
                            scalar=1.0, in1=rb[:, i, :qn],
                            op0=mybir.AluOpType.mult,
                            op1=mybir.AluOpType.mult)
                    else:
                        otmp = otmp_pool.tile([64, 512], DT, tag="otmp",
                                              name="otmp")
                        nc.vector.scalar_tensor_tensor(
                            out=otmp[:, :qn], in0=ot_ps[h][0:64, :qn],
                            scalar=1.0, in1=rb[:, i, :qn],
                            op0=mybir.AluOpType.mult,
                            op1=mybir.AluOpType.mult)
                        nc.sync.dma_start(osbp[64:128, :qn], otmp[:, :qn])

            for jt in range((qn + 127) // 128):
                jobs.append(
                    lambda a=q0, b=qn, j=jt, o=tuple(osb_c):
                    y_job(a, b, j, o))
        drain(len(jobs))
    nc.compile()
    return nc


_nc_cache: dict = {}


def _get_nc(SA: int):
    key = (SA, MM_DTYPE)
    if key not in _nc_cache:
        _nc_cache[key] = _build(SA, MM_DTYPE)
    return _nc_cache[key]


def _reference_fallback(x, gate, Wq, bq, Wk, bk, Wv, bv, Wo, bo):
    g = gate.astype(x.dtype)[..., None]
    q = (x @ Wq + bq) * g
    k = (x @ Wk + bk) * g
    v = (x @ Wv + bv) * g

    def split(t):
        return t.reshape(B, S, H, DH).transpose(0, 2, 1, 3)

    q, k, v = split(q), split(k), split(v)
    sc = np.einsum('bhqd,bhkd->bhqk', q, k) / np.float32(math.sqrt(DH))
    sc = sc - sc.max(axis=-1, keepdims=True)
    e = np.exp(sc)
    attn = e / e.sum(axis=-1, keepdims=True)
    out = np.einsum('bhqk,bhkd->bhqd', attn, v)
    out = out.transpose(0, 2, 1, 3).reshape(B, S, D)
    out = out @ Wo + bo
    return (x * (1.0 - g) + out * g).astype(np.float32)


def kernel(x, gate, Wq, bq, Wk, bk, Wv, bv, Wo, bo, _profile=None):
    x = np.asarray(x, np.float32)
    gate = np.asarray(gate)
    args = dict(x=x, gate=gate, Wq=np.asarray(Wq, np.float32),
                bq=np.asarray(bq, np.float32), Wk=np.asarray(Wk, np.float32),
                bk=np.asarray(bk, np.float32), Wv=np.asarray(Wv, np.float32),
                bv=np.asarray(bv, np.float32), Wo=np.asarray(Wo, np.float32),
                bo=np.asarray(bo, np.float32))

    idxs = [np.nonzero(gate[b])[0] for b in range(B)]
    n_act = [len(i) for i in idxs]
    # the compaction trick needs zero q/k/v biases and at least one active
    # and one inactive token per batch; otherwise fall back to exact numpy
    if (any(np.abs(args[k]).max() > 0 for k in ("bq", "bk", "bv"))
            or min(n_act) == 0 or max(n_act) == S):
        return _reference_fallback(**args)

    SA = ((max(n_act) + 127) // 128) * 128
    npdt = _NPDT[MM_DTYPE]

    in_maps = []
    for b in range(B):
        xa = np.zeros((SA, D), np.float32)
        xa[:n_act[b]] = x[b, idxs[b]]
        xT = np.ascontiguousarray(xa.T).astype(npdt)
        for g in range(GROUPS):
            cs = slice(g * DC, (g + 1) * DC)
            in_maps.append({
                "xT": xT,
                "wq": np.ascontiguousarray(args["Wq"][:, cs]).astype(npdt),
                "wk": np.ascontiguousarray(args["Wk"][:, cs]).astype(npdt),
                "wv": np.ascontiguousarray(args["Wv"][:, cs]).astype(npdt),
                "wo": np.ascontiguousarray(args["Wo"][cs, :]).astype(npdt),
            })

    nc = _get_nc(SA)
    kw = dict(_profile) if _profile else {}
    kw.pop("result", None)
    res = run_bass_kernel_spmd(nc, in_maps, core_ids=list(range(NCORES)), **kw)
    if _profile is not None:
        _profile["result"] = res

    out = x.copy()
    for b in range(B):
        Y = np.zeros((SA, D), np.float32)
        for g in range(GROUPS):
            Y += np.asarray(res.results[b * GROUPS + g]["y"]).astype(np.float32)
        out[b, idxs[b]] = Y[:n_act[b]] + args["bo"]
    return out


# revision 8
# speedup vs baseline: 1.0128x; 1.0128x over previous
"""Entropy-gated multi-head attention on 8 Trainium2 NeuronCores.

Sharding: core c = b*4 + g handles batch b (of 2) and head-group g (4 of the
16 heads).  Tokens with gate==0 pass x through untouched and contribute
exactly zero k/v (zero biases), so the device only processes the compacted
active tokens (~half), with the softmax denominator corrected by the count of
inactive tokens: each inactive key contributes exp(0)=1 to the softmax sum
(scores vs. zeroed k are exactly 0) and nothing to the numerator (v=0).

Device math per core (no max-subtraction; scores are O(5) so exp is safe):
  QT = Wq_g^T x^T, KT = Wk_g^T x^T           [256, SA]
  V  = x Wv_g                                 [SA, 256]
  per head h: PT = exp((KT_h^T QT_h)/8)       [SA_k, SA_q]
              OT' = [V_h | 1]^T PT            [65, SA_q] (row 64 = colsum = Z_act)
              r = 1/(Z_act + (S - SA))        recip on [2,qn] THEN broadcast
              osb_h = OT * r                  (scaled attention out, lhsT form)
  Y(q, :) = sum_h osb_h^T Wo_h                (pair-packed K=128 matmuls)
Host sums the 4 per-group partial Y per batch, adds bo, scatters into x.

v2 changes vs baseline (183976ns):
  - Y matmuls pair-packed K=128 (half the Y instruction cycles)
  - reciprocal_approx_fast on [2,qn] pre-broadcast (was reciprocal on
    [64,qn] post-broadcast: ~30us of DVE)
  - CADD folded into the same small pre-broadcast op
  - y output in bf16 (half the output DMA)
  - s-granular software pipeline: scores(s) / exp(s) / AV(s-1) with filler
    jobs drained every round to keep the PE continuously busy (max p-state)
  - PSUM: st bufs=2 (1 bank each), ot bufs=4, aux bufs=2 -> exactly 8 banks
"""

import math
from contextlib import ExitStack

import numpy as np
import ml_dtypes

import concourse.bass as bass
import concourse.mybir as mybir
from concourse import bacc
import concourse.tile as tile
from concourse.bass_utils import run_bass_kernel_spmd

B, S, D = 2, 2048, 1024
H, DH = 16, 64
NCORES = 8
GROUPS = NCORES // B          # head-groups per batch = 4
HC = H // GROUPS              # heads per core = 4
DC = HC * DH                  # head-group width = 256

MM_DTYPE = "bf16"

_DT = {"bf16": mybir.dt.bfloat16, "f32": mybir.dt.float32}
_NPDT = {"bf16": ml_dtypes.bfloat16, "f32": np.float32}

f32 = mybir.dt.float32
bf16 = mybir.dt.bfloat16


def _chunks(total, step):
    out = []
    o = 0
    while o < total:
        out.append((o, min(step, total - o)))
        o += step
    return out


def _build(SA: int, dtype_tag: str) -> bass.Bass:
    DT = _DT[dtype_tag]
    nkt = D // 128            # 8 contraction tiles for projections
    nst = SA // 128           # token (key) tiles
    qch = _chunks(SA, 512)    # q chunks
    dch = _chunks(D, 512)     # output-dim chunks
    CADD = float(S - SA)      # inactive keys not represented by padding

    nc = bacc.Bacc()
    xT_d = nc.dram_tensor("xT", [D, SA], DT, kind="ExternalInput")
    wq_d = nc.dram_tensor("wq", [D, DC], DT, kind="ExternalInput")
    wk_d = nc.dram_tensor("wk", [D, DC], DT, kind="ExternalInput")
    wv_d = nc.dram_tensor("wv", [D, DC], DT, kind="ExternalInput")
    wo_d = nc.dram_tensor("wo", [DC, D], DT, kind="ExternalInput")
    y_d = nc.dram_tensor("y", [SA, D], bf16, kind="ExternalOutput")

    with tile.TileContext(nc) as tc, ExitStack() as ctx:
        singles = ctx.enter_context(tc.tile_pool(name="singles", bufs=1))
        pt_pool = ctx.enter_context(tc.tile_pool(name="pt", bufs=3))
        osb_pool = ctx.enter_context(tc.tile_pool(name="osb", bufs=3))
        otmp_pool = ctx.enter_context(tc.tile_pool(name="otmp", bufs=2))
        zsb_pool = ctx.enter_context(tc.tile_pool(name="zsb", bufs=2))
        rbc_pool = ctx.enter_context(tc.tile_pool(name="rbc", bufs=3))
        yout_pool = ctx.enter_context(tc.tile_pool(name="yout", bufs=3))
        zscr_pool = ctx.enter_context(tc.tile_pool(name="zscr", bufs=2,
                                                   space="DRAM"))
        # PSUM budget: st(pair) 2x2 + ot 3x1 + aux 1x1 = 8 banks
        st_ps = ctx.enter_context(tc.tile_pool(name="stps", bufs=2, space="PSUM"))
        ot_ps_pool = ctx.enter_context(tc.tile_pool(name="otps", bufs=3, space="PSUM"))
        aux_ps = ctx.enter_context(tc.tile_pool(name="auxps", bufs=1, space="PSUM"))

        # ---- persistent SBUF; DMA order = consumption order for fast start
        wk_sb = singles.tile([128, nkt, DC], DT)
        wq_sb = singles.tile([128, nkt, DC], DT)
        wv_sb = singles.tile([128, nkt, DC], DT)
        xt = singles.tile([128, nkt, SA], DT)
        (c0, c0n) = qch[0]
        for t in range(nkt):
            nc.sync.dma_start(wk_sb[:, t, :], wk_d[t * 128:(t + 1) * 128, :])
            nc.sync.dma_start(xt[:, t, c0:c0 + c0n],
                              xT_d[t * 128:(t + 1) * 128, c0:c0 + c0n])
            nc.sync.dma_start(wq_sb[:, t, :], wq_d[t * 128:(t + 1) * 128, :])
        for t in range(nkt):
            nc.sync.dma_start(wv_sb[:, t, :], wv_d[t * 128:(t + 1) * 128, :])
        for (q0, qn) in qch[1:]:
            for t in range(nkt):
                nc.sync.dma_start(xt[:, t, q0:q0 + qn],
                                  xT_d[t * 128:(t + 1) * 128, q0:q0 + qn])
        wo_sb = []
        for p in range(HC // 2):
            w = singles.tile([128, D], DT, tag=f"wo{p}", name=f"wo{p}")
            nc.sync.dma_start(w, wo_d[p * 128:(p + 1) * 128, :])
            wo_sb.append(w)

        # ---- projections: QT/KT [256, SA], V (augmented with ones) ----
        qt = [singles.tile([128, SA], DT, tag=f"qt{m}", name=f"qt{m}")
              for m in range(2)]
        kt = [singles.tile([128, SA], DT, tag=f"kt{m}", name=f"kt{m}")
              for m in range(2)]
        v_aug = singles.tile([128, nst, HC, 65], DT)

        # ACT exp-table preload: tiny dummy exp as the very first ACT
        # instruction so the ~1.3us table load overlaps the input DMAs.
        dummy = singles.tile([1, 16], f32, tag="dummy", name="dummy")
        nc.vector.memset(dummy, 0.0)
        nc.scalar.activation(dummy, dummy,
                             mybir.ActivationFunctionType.Exp, scale=1.0)
        nc.vector.memset(v_aug[:, :, :, 64:65], 1.0)

        def proj_qk(m, dst, w_sb, q0, qn):
            ps = aux_ps.tile([128, 512], f32, tag="aux", name="ps")
            for t in range(nkt):
                nc.tensor.matmul(
                    ps[:, :qn],
                    w_sb[:, t, m * 128:(m + 1) * 128],
                    xt[:, t, q0:q0 + qn],
                    start=(t == 0), stop=(t == nkt - 1))
            nc.vector.tensor_copy(dst[m][:, q0:q0 + qn], ps[:, :qn])

        v_done = set()

        def proj_v(s):
            v_done.add(s)
            ps = aux_ps.tile([128, 512], f32, tag="aux", name="ps")
            for t in range(nkt):
                nc.tensor.matmul(
                    ps[:, :DC],
                    xt[:, t, s * 128:(s + 1) * 128],
                    wv_sb[:, t, :],
                    start=(t == 0), stop=(t == nkt - 1))
            for h in range(HC):
                nc.vector.tensor_copy(v_aug[:, s, h, 0:64],
                                      ps[:, h * 64:(h + 1) * 64])

        # filler queue: independent PE work drained between pipeline rounds
        jobs = []

        def drain(k):
            for _ in range(min(k, len(jobs))):
                jobs.pop(0)()

        # warmup: only what chunk0/pair0 strictly needs, rest queued
        proj_qk(0, kt, wk_sb, c0, c0n)
        proj_qk(0, qt, wq_sb, c0, c0n)
        proj_v(0)
        proj_v(1)
        for s in range(2, nst):
            jobs.append(lambda s=s: proj_v(s))
        jobs.append(lambda: proj_qk(1, kt, wk_sb, c0, c0n))
        jobs.append(lambda: proj_qk(1, qt, wq_sb, c0, c0n))
        for (q0, qn) in qch[1:]:
            jobs.append(lambda a=q0, b=qn: proj_qk(0, kt, wk_sb, a, b))
            jobs.append(lambda a=q0, b=qn: proj_qk(0, qt, wq_sb, a, b))
            jobs.append(lambda a=q0, b=qn: proj_qk(1, kt, wk_sb, a, b))
            jobs.append(lambda a=q0, b=qn: proj_qk(1, qt, wq_sb, a, b))

        # output projection: pair-packed K=128 accumulating matmuls
        def y_job(q0, qn, jt, osb_pair):
            qtn = min(128, qn - jt * 128)
            for (d0, dn) in dch:
                yp = aux_ps.tile([128, 512], f32, tag="aux", name="yp")
                for p in range(HC // 2):
                    nc.tensor.matmul(
                        yp[:qtn, :dn],
                        osb_pair[p][:, jt * 128:jt * 128 + qtn],
                        wo_sb[p][:, d0:d0 + dn],
                        start=(p == 0), stop=(p == HC // 2 - 1))
                yo = yout_pool.tile([128, 512], bf16, tag="yo", name="yo")
                nc.vector.tensor_copy(yo[:qtn, :dn], yp[:qtn, :dn])
                nc.sync.dma_start(
                    y_d[q0 + jt * 128: q0 + jt * 128 + qtn, d0:d0 + dn],
                    yo[:qtn, :dn])

        # ---- attention: s-granular pipeline per (chunk, pair) ----
        for ci, (q0, qn) in enumerate(qch):
            osb_c = [None, None]
            for p in range(HC // 2):
                m = p
                h0, h1 = 2 * p, 2 * p + 1
                ot_ps = {h: ot_ps_pool.tile([65, 512], f32, tag="ot",
                                            name="ot_ps")
                         for h in (h0, h1)}
                pt_prev = None
                for s in range(nst):
                    # filler first: keeps PE busy while ACT runs exp(s-1)
                    drain(2 if (ci == 0 and p == 0 and s < 6) else 1)
                    # both heads share one 2-bank st tile -> one exp instr
                    st = st_ps.tile([128, 2, 512], f32, tag="st",
                                    name="st_ps")
                    for i, h in enumerate((h0, h1)):
                        r0 = (h % 2) * 64
                        nc.tensor.matmul(
                            st[:, i, :qn],
                            kt[m][r0:r0 + 64, s * 128:(s + 1) * 128],
                            qt[m][r0:r0 + 64, q0:q0 + qn],
                            start=True, stop=True,
                            tile_position=(r0, 0))
                    pt = pt_pool.tile([128, 2, 512], DT, tag="pt", name="pt")
                    nc.scalar.activation(
                        pt[:, :, :qn], st[:, :, :qn],
                        mybir.ActivationFunctionType.Exp, scale=0.125)
                    # AV lagged one round so exp(s-1) is done when PE gets here
                    if pt_prev is not None:
                        sp = s - 1
                        assert sp in v_done, \
                            f"proj_v not emitted before OT consumer: {sp}"
                        for i, h in enumerate((h0, h1)):
                            nc.tensor.matmul(
                                ot_ps[h][:, :qn],
                                v_aug[:, sp, h, :],
                                pt_prev[:, i, :qn],
                                start=(sp == 0), stop=False)
                    pt_prev = pt
                drain(1)
                sp = nst - 1
                assert sp in v_done
                for i, h in enumerate((h0, h1)):
                    nc.tensor.matmul(
                        ot_ps[h][:, :qn],
                        v_aug[:, sp, h, :],
                        pt_prev[:, i, :qn],
                        start=(sp == 0), stop=True)

                # Z path, per head (so h0's chain never waits on h1):
                # DVE +CADD (lane 64) -> DMA to partition 0 -> fast approx
                # reciprocal (the custom DVE op silently no-ops on nonzero
                # partition offsets, hence the partition hop) -> DRAM hop ->
                # partition-broadcast -> scale.
                osbp = osb_pool.tile([128, 512], DT, tag="osbp", name="osbp")
                osb_c[p] = osbp
                for i, h in enumerate((h0, h1)):
                    zt = zsb_pool.tile([65, 512], f32, tag="zt", name="zt")
                    nc.vector.tensor_scalar(
                        out=zt[64:65, :qn], in0=ot_ps[h][64:65, :qn],
                        scalar1=CADD, scalar2=None, op0=mybir.AluOpType.add)
                    zlow = zsb_pool.tile([1, 512], f32, tag="zlow",
                                         name="zlow")
                    nc.sync.dma_start(zlow[0:1, :qn], zt[64:65, :qn])
                    nc.vector.reciprocal_approx_fast(zlow[0:1, :qn],
                                                     zlow[0:1, :qn])
                    zd = zscr_pool.tile([1, 512], f32, tag="zd", name="zd")
                    nc.sync.dma_start(zd[0:1, :qn], zlow[0:1, :qn])
                    rb = rbc_pool.tile([64, 512], f32, tag="rb", name="rb")
                    nc.sync.dma_start(rb[:, :qn],
                                      zd[0:1, :qn].to_broadcast((64, qn)))
                    if i == 0:
                        nc.vector.scalar_tensor_tensor(
                            out=osbp[0:64, :qn], in0=ot_ps[h][0:64, :qn],
                            scalar=1.0, in1=rb[:, :qn],
                            op0=mybir.AluOpType.mult,
                            op1=mybir.AluOpType.mult)
                    else:
                        otmp = otmp_pool.tile([64, 512], DT, tag="otmp",
                                              name="otmp")
                        nc.vector.scalar_tensor_tensor(
                            out=otmp[:, :qn], in0=ot_ps[h][0:64, :qn],
                            scalar=1.0, in1=rb[:, :qn],
                            op0=mybir.AluOpType.mult,
                            op1=mybir.AluOpType.mult)
                        nc.sync.dma_start(osbp[64:128, :qn], otmp[:, :qn])

            for jt in range((qn + 127) // 128):
                jobs.append(
                    lambda a=q0, b=qn, j=jt, o=tuple(osb_c):
                    y_job(a, b, j, o))
        drain(len(jobs))
    nc.compile()
    return nc


_nc_cache: dict = {}


def _get_nc(SA: int):
    key = (SA, MM_DTYPE)
    if key not in _nc_cache:
        _nc_cache[key] = _build(SA, MM_DTYPE)
    return _nc_cache[key]


def _reference_fallback(x, gate, Wq, bq, Wk, bk, Wv, bv, Wo, bo):
    g = gate.astype(x.dtype)[..., None]
    q = (x @ Wq + bq) * g
    k = (x @ Wk + bk) * g
    v = (x @ Wv + bv) * g

    def split(t):
        return t.reshape(B, S, H, DH).transpose(0, 2, 1, 3)

    q, k, v = split(q), split(k), split(v)
    sc = np.einsum('bhqd,bhkd->bhqk', q, k) / np.float32(math.sqrt(DH))
    sc = sc - sc.max(axis=-1, keepdims=True)
    e = np.exp(sc)
    attn = e / e.sum(axis=-1, keepdims=True)
    out = np.einsum('bhqk,bhkd->bhqd', attn, v)
    out = out.transpose(0, 2, 1, 3).reshape(B, S, D)
    out = out @ Wo + bo
    return (x * (1.0 - g) + out * g).astype(np.float32)


def kernel(x, gate, Wq, bq, Wk, bk, Wv, bv, Wo, bo, _profile=None):
    x = np.asarray(x, np.float32)
    gate = np.asarray(gate)
    args = dict(x=x, gate=gate, Wq=np.asarray(Wq, np.float32),
                bq=np.asarray(bq, np.float32), Wk=np.asarray(Wk, np.float32),
                bk=np.asarray(bk, np.float32), Wv=np.asarray(Wv, np.float32),
                bv=np.asarray(bv, np.float32), Wo=np.asarray(Wo, np.float32),
                bo=np.asarray(bo, np.float32))

    idxs = [np.nonzero(gate[b])[0] for b in range(B)]
    n_act = [len(i) for i in idxs]
    # the compaction trick needs zero q/k/v biases and at least one active
    # and one inactive token per batch; otherwise fall back to exact numpy
    if (any(np.abs(args[k]).max() > 0 for k in ("bq", "bk", "bv"))
            or min(n_act) == 0 or max(n_act) == S):
        return _reference_fallback(**args)

    SA = ((max(n_act) + 127) // 128) * 128
    npdt = _NPDT[MM_DTYPE]

    in_maps = []
    for b in range(B):
        xa = np.zeros((SA, D), np.float32)
        xa[:n_act[b]] = x[b, idxs[b]]
        xT = np.ascontiguousarray(xa.T).astype(npdt)
        for g in range(GROUPS):
            cs = slice(g * DC, (g + 1) * DC)
            in_maps.append({
                "xT": xT,
                "wq": np.ascontiguousarray(args["Wq"][:, cs]).astype(npdt),
                "wk": np.ascontiguousarray(args["Wk"][:, cs]).astype(npdt),
                "wv": np.ascontiguousarray(args["Wv"][:, cs]).astype(npdt),
                "wo": np.ascontiguousarray(args["Wo"][cs, :]).astype(npdt),
            })

    nc = _get_nc(SA)
    kw = dict(_profile) if _profile else {}
    kw.pop("result", None)
    res = run_bass_kernel_spmd(nc, in_maps, core_ids=list(range(NCORES)), **kw)
    if _profile is not None:
        _profile["result"] = res

    out = x.copy()
    for b in range(B):
        Y = np.zeros((SA, D), np.float32)
        for g in range(GROUPS):
            Y += np.asarray(res.results[b * GROUPS + g]["y"]).astype(np.float32)
        out[b, idxs[b]] = Y[:n_act[b]] + args["bo"]
    return out


# revision 14
# speedup vs baseline: 1.2443x; 1.2285x over previous
"""Entropy-gated multi-head attention on 8 Trainium2 NeuronCores.

Sharding: core c = b*4 + g handles batch b (of 2) and head-group g (4 of the
16 heads).  Tokens with gate==0 pass x through untouched and contribute
exactly zero k/v (zero biases), so the device only processes the compacted
active tokens (~half), with the softmax denominator corrected by the count of
inactive tokens: each inactive key contributes exp(0)=1 to the softmax sum
(scores vs. zeroed k are exactly 0) and nothing to the numerator (v=0).

Device math per core (no max-subtraction; scores are O(5) so exp is safe):
  QT = Wq_g^T x^T, KT = Wk_g^T x^T           [256, SA]
  V  = x Wv_g                                 [SA, 256]
  per head h: PT = exp((KT_h^T QT_h)/8)       [SA_k, SA_q]
              OT' = [V_h | 1]^T PT            [65, SA_q] (row 64 = colsum = Z_act)
              r = 1/(Z_act + (S - SA))        recip on [2,qn] THEN broadcast
              osb_h = OT * r                  (scaled attention out, lhsT form)
  Y(q, :) = sum_h osb_h^T Wo_h                (pair-packed K=128 matmuls)
Host sums the 4 per-group partial Y per batch, adds bo, scatters into x.

v2 changes vs baseline (183976ns):
  - Y matmuls pair-packed K=128 (half the Y instruction cycles)
  - reciprocal_approx_fast on [2,qn] pre-broadcast (was reciprocal on
    [64,qn] post-broadcast: ~30us of DVE)
  - CADD folded into the same small pre-broadcast op
  - y output in bf16 (half the output DMA)
  - s-granular software pipeline: scores(s) / exp(s) / AV(s-1) with filler
    jobs drained every round to keep the PE continuously busy (max p-state)
  - PSUM: st bufs=2 (1 bank each), ot bufs=4, aux bufs=2 -> exactly 8 banks
"""

import math
from contextlib import ExitStack

import numpy as np
import ml_dtypes

import concourse.bass as bass
import concourse.mybir as mybir
from concourse import bacc
import concourse.tile as tile
from concourse.bass_utils import run_bass_kernel_spmd

B, S, D = 2, 2048, 1024
H, DH = 16, 64
NCORES = 8
GROUPS = NCORES // B          # head-groups per batch = 4
HC = H // GROUPS              # heads per core = 4
DC = HC * DH                  # head-group width = 256

MM_DTYPE = "bf16"

_DT = {"bf16": mybir.dt.bfloat16, "f32": mybir.dt.float32}
_NPDT = {"bf16": ml_dtypes.bfloat16, "f32": np.float32}

f32 = mybir.dt.float32
bf16 = mybir.dt.bfloat16


def _chunks(total, step):
    out = []
    o = 0
    while o < total:
        out.append((o, min(step, total - o)))
        o += step
    return out


def _build(SA: int, dtype_tag: str) -> bass.Bass:
    DT = _DT[dtype_tag]
    nkt = D // 128            # 8 contraction tiles for projections
    nst = SA // 128           # token (key) tiles
    qch = _chunks(SA, 512)    # q chunks
    dch = _chunks(D, 512)     # output-dim chunks
    CADD = float(S - SA)      # inactive keys not represented by padding

    nc = bacc.Bacc()
    xT_d = nc.dram_tensor("xT", [D, SA], DT, kind="ExternalInput")
    wq_d = nc.dram_tensor("wq", [D, DC], DT, kind="ExternalInput")
    wk_d = nc.dram_tensor("wk", [D, DC], DT, kind="ExternalInput")
    wv_d = nc.dram_tensor("wv", [D, DC], DT, kind="ExternalInput")
    wo_d = nc.dram_tensor("wo", [DC, D], DT, kind="ExternalInput")
    y_d = nc.dram_tensor("y", [SA, D], bf16, kind="ExternalOutput")

    with tile.TileContext(nc) as tc, ExitStack() as ctx:
        singles = ctx.enter_context(tc.tile_pool(name="singles", bufs=1))
        pt_pool = ctx.enter_context(tc.tile_pool(name="pt", bufs=3))
        osb_pool = ctx.enter_context(tc.tile_pool(name="osb", bufs=3))
        otmp_pool = ctx.enter_context(tc.tile_pool(name="otmp", bufs=2))
        zsb_pool = ctx.enter_context(tc.tile_pool(name="zsb", bufs=2))
        rbc_pool = ctx.enter_context(tc.tile_pool(name="rbc", bufs=3))
        yout_pool = ctx.enter_context(tc.tile_pool(name="yout", bufs=3))
        zscr_pool = ctx.enter_context(tc.tile_pool(name="zscr", bufs=2,
                                                   space="DRAM"))
        # PSUM budget: st(pair) 1x2 + ot 4x1 + aux 2x1 = 8 banks
        st_ps = ctx.enter_context(tc.tile_pool(name="stps", bufs=1, space="PSUM"))
        ot_ps_pool = ctx.enter_context(tc.tile_pool(name="otps", bufs=4, space="PSUM"))
        aux_ps = ctx.enter_context(tc.tile_pool(name="auxps", bufs=2, space="PSUM"))

        # ---- persistent SBUF; DMA order = consumption order for fast start
        wk_sb = singles.tile([128, nkt, DC], DT)
        wq_sb = singles.tile([128, nkt, DC], DT)
        wv_sb = singles.tile([128, nkt, DC], DT)
        xt = singles.tile([128, nkt, SA], DT)
        (c0, c0n) = qch[0]
        for t in range(nkt):
            nc.sync.dma_start(wk_sb[:, t, :], wk_d[t * 128:(t + 1) * 128, :])
            nc.sync.dma_start(xt[:, t, c0:c0 + c0n],
                              xT_d[t * 128:(t + 1) * 128, c0:c0 + c0n])
            nc.sync.dma_start(wq_sb[:, t, :], wq_d[t * 128:(t + 1) * 128, :])
        for t in range(nkt):
            nc.sync.dma_start(wv_sb[:, t, :], wv_d[t * 128:(t + 1) * 128, :])
        for (q0, qn) in qch[1:]:
            for t in range(nkt):
                nc.sync.dma_start(xt[:, t, q0:q0 + qn],
                                  xT_d[t * 128:(t + 1) * 128, q0:q0 + qn])
        wo_sb = []
        for p in range(HC // 2):
            w = singles.tile([128, D], DT, tag=f"wo{p}", name=f"wo{p}")
            nc.sync.dma_start(w, wo_d[p * 128:(p + 1) * 128, :])
            wo_sb.append(w)

        # ---- projections: QT/KT [256, SA], V (augmented with ones) ----
        qt = [singles.tile([128, SA], DT, tag=f"qt{m}", name=f"qt{m}")
              for m in range(2)]
        kt = [singles.tile([128, SA], DT, tag=f"kt{m}", name=f"kt{m}")
              for m in range(2)]
        v_aug = singles.tile([128, nst, HC, 65], DT)

        # ACT exp-table preload: tiny dummy exp as the very first ACT
        # instruction so the ~1.3us table load overlaps the input DMAs.
        dummy = singles.tile([1, 16], f32, tag="dummy", name="dummy")
        nc.vector.memset(dummy, 0.0)
        nc.scalar.activation(dummy, dummy,
                             mybir.ActivationFunctionType.Exp, scale=1.0)
        # ones column FIRST: the AV output then has Z on psum partition 0,
        # where the (partition-offset-buggy) fast reciprocal can run without
        # a partition-relocating DMA hop.  V lives in columns 1:65; the host
        # pre-rotates each 128-row block of Wo by +1 to match the shifted
        # row layout of osbp.
        nc.vector.memset(v_aug[:, :, :, 0:1], 1.0)

        def proj_qk(m, dst, w_sb, q0, qn):
            ps = aux_ps.tile([128, 512], f32, tag="aux", name="ps")
            for t in range(nkt):
                nc.tensor.matmul(
                    ps[:, :qn],
                    w_sb[:, t, m * 128:(m + 1) * 128],
                    xt[:, t, q0:q0 + qn],
                    start=(t == 0), stop=(t == nkt - 1))
            nc.vector.tensor_copy(dst[m][:, q0:q0 + qn], ps[:, :qn])

        v_done = set()

        def proj_v(s):
            v_done.add(s)
            ps = aux_ps.tile([128, 512], f32, tag="aux", name="ps")
            for t in range(nkt):
                nc.tensor.matmul(
                    ps[:, :DC],
                    xt[:, t, s * 128:(s + 1) * 128],
                    wv_sb[:, t, :],
                    start=(t == 0), stop=(t == nkt - 1))
            for h in range(HC):
                nc.vector.tensor_copy(v_aug[:, s, h, 1:65],
                                      ps[:, h * 64:(h + 1) * 64])

        # filler queue: independent PE work drained between pipeline rounds
        jobs = []

        def drain(k):
            for _ in range(min(k, len(jobs))):
                jobs.pop(0)()

        # warmup: only what chunk0/pair0 strictly needs, rest queued
        proj_qk(0, kt, wk_sb, c0, c0n)
        proj_qk(0, qt, wq_sb, c0, c0n)
        proj_v(0)
        proj_v(1)
        for s in range(2, nst):
            jobs.append(lambda s=s: proj_v(s))
        jobs.append(lambda: proj_qk(1, kt, wk_sb, c0, c0n))
        jobs.append(lambda: proj_qk(1, qt, wq_sb, c0, c0n))
        for (q0, qn) in qch[1:]:
            jobs.append(lambda a=q0, b=qn: proj_qk(0, kt, wk_sb, a, b))
            jobs.append(lambda a=q0, b=qn: proj_qk(0, qt, wq_sb, a, b))
            jobs.append(lambda a=q0, b=qn: proj_qk(1, kt, wk_sb, a, b))
            jobs.append(lambda a=q0, b=qn: proj_qk(1, qt, wq_sb, a, b))

        # output projection: pair-packed K=128 accumulating matmuls
        def y_job(q0, qn, jt, osb_pair):
            qtn = min(128, qn - jt * 128)
            for (d0, dn) in dch:
                yp = aux_ps.tile([128, 512], f32, tag="aux", name="yp")
                for p in range(HC // 2):
                    nc.tensor.matmul(
                        yp[:qtn, :dn],
                        osb_pair[p][:, jt * 128:jt * 128 + qtn],
                        wo_sb[p][:, d0:d0 + dn],
                        start=(p == 0), stop=(p == HC // 2 - 1))
                yo = yout_pool.tile([128, 512], bf16, tag="yo", name="yo")
                nc.vector.tensor_copy(yo[:qtn, :dn], yp[:qtn, :dn])
                nc.sync.dma_start(
                    y_d[q0 + jt * 128: q0 + jt * 128 + qtn, d0:d0 + dn],
                    yo[:qtn, :dn])

        # ---- attention: s-granular pipeline per (chunk, pair) ----
        for ci, (q0, qn) in enumerate(qch):
            osb_c = [None, None]
            for p in range(HC // 2):
                m = p
                h0, h1 = 2 * p, 2 * p + 1
                ot_ps = {h: ot_ps_pool.tile([65, 512], f32, tag="ot",
                                            name="ot_ps")
                         for h in (h0, h1)}
                pt_prev = None
                for s in range(nst):
                    # filler first: keeps PE busy while ACT runs exp(s-1)
                    drain(2 if (ci == 0 and p == 0 and s < 6) else 1)
                    # both heads share one 2-bank st tile -> one exp instr
                    st = st_ps.tile([128, 2, 512], f32, tag="st",
                                    name="st_ps")
                    for i, h in enumerate((h0, h1)):
                        r0 = (h % 2) * 64
                        nc.tensor.matmul(
                            st[:, i, :qn],
                            kt[m][r0:r0 + 64, s * 128:(s + 1) * 128],
                            qt[m][r0:r0 + 64, q0:q0 + qn],
                            start=True, stop=True,
                            tile_position=(r0, 0))
                    pt = pt_pool.tile([128, 2, 512], DT, tag="pt", name="pt")
                    nc.scalar.activation(
                        pt[:, :, :qn], st[:, :, :qn],
                        mybir.ActivationFunctionType.Exp, scale=0.125)
                    # AV lagged one round so exp(s-1) is done when PE gets here
                    if pt_prev is not None:
                        sp = s - 1
                        assert sp in v_done, \
                            f"proj_v not emitted before OT consumer: {sp}"
                        for i, h in enumerate((h0, h1)):
                            nc.tensor.matmul(
                                ot_ps[h][:, :qn],
                                v_aug[:, sp, h, :],
                                pt_prev[:, i, :qn],
                                start=(sp == 0), stop=False)
                    pt_prev = pt
                drain(1)
                sp = nst - 1
                assert sp in v_done
                for i, h in enumerate((h0, h1)):
                    nc.tensor.matmul(
                        ot_ps[h][:, :qn],
                        v_aug[:, sp, h, :],
                        pt_prev[:, i, :qn],
                        start=(sp == 0), stop=True)

                # Z path, per head (so h0's chain never waits on h1):
                # Z sits on psum partition 0 -> +CADD and fast reciprocal
                # run back-to-back on DVE at partition 0 (no relocation DMA),
                # then DRAM hop -> partition-broadcast -> scale.
                # osbp row layout (host pre-rotates Wo blocks to match):
                #   row 0      = h1 hd 127
                #   rows 1-64  = h0 hd 0-63
                #   rows 65-127= h1 hd 64-126
                osbp = osb_pool.tile([128, 512], DT, tag="osbp", name="osbp")
                osb_c[p] = osbp
                for i, h in enumerate((h0, h1)):
                    zq = zsb_pool.tile([1, 512], f32, tag="zq", name="zq")
                    nc.vector.tensor_scalar(
                        out=zq[0:1, :qn], in0=ot_ps[h][0:1, :qn],
                        scalar1=CADD, scalar2=None, op0=mybir.AluOpType.add)
                    nc.vector.reciprocal_approx_fast(zq[0:1, :qn],
                                                     zq[0:1, :qn])
                    zd = zscr_pool.tile([1, 512], f32, tag="zd", name="zd")
                    nc.sync.dma_start(zd[0:1, :qn], zq[0:1, :qn])
                    rb = rbc_pool.tile([65, 512], f32, tag="rb", name="rb")
                    nc.sync.dma_start(rb[0:65, :qn],
                                      zd[0:1, :qn].to_broadcast((65, qn)))
                    # STT covers rows 0-64 (start partition must be 0/32/64);
                    # row 0 is Z*r garbage, overwritten by the h1 relocate.
                    if i == 0:
                        nc.vector.scalar_tensor_tensor(
                            out=osbp[0:65, :qn], in0=ot_ps[h][0:65, :qn],
                            scalar=1.0, in1=rb[0:65, :qn],
                            op0=mybir.AluOpType.mult,
                            op1=mybir.AluOpType.mult)
                    else:
                        otmp = otmp_pool.tile([65, 512], DT, tag="otmp",
                                              name="otmp")
                        nc.vector.scalar_tensor_tensor(
                            out=otmp[0:65, :qn], in0=ot_ps[h][0:65, :qn],
                            scalar=1.0, in1=rb[0:65, :qn],
                            op0=mybir.AluOpType.mult,
                            op1=mybir.AluOpType.mult)
                        nc.sync.dma_start(osbp[65:128, :qn],
                                          otmp[1:64, :qn])
                        nc.sync.dma_start(osbp[0:1, :qn],
                                          otmp[64:65, :qn])

            for jt in range((qn + 127) // 128):
                jobs.append(
                    lambda a=q0, b=qn, j=jt, o=tuple(osb_c):
                    y_job(a, b, j, o))
        drain(len(jobs))
    nc.compile()
    return nc


_nc_cache: dict = {}


def _get_nc(SA: int):
    key = (SA, MM_DTYPE)
    if key not in _nc_cache:
        _nc_cache[key] = _build(SA, MM_DTYPE)
    return _nc_cache[key]


def _reference_fallback(x, gate, Wq, bq, Wk, bk, Wv, bv, Wo, bo):
    g = gate.astype(x.dtype)[..., None]
    q = (x @ Wq + bq) * g
    k = (x @ Wk + bk) * g
    v = (x @ Wv + bv) * g

    def split(t):
        return t.reshape(B, S, H, DH).transpose(0, 2, 1, 3)

    q, k, v = split(q), split(k), split(v)
    sc = np.einsum('bhqd,bhkd->bhqk', q, k) / np.float32(math.sqrt(DH))
    sc = sc - sc.max(axis=-1, keepdims=True)
    e = np.exp(sc)
    attn = e / e.sum(axis=-1, keepdims=True)
    out = np.einsum('bhqk,bhkd->bhqd', attn, v)
    out = out.transpose(0, 2, 1, 3).reshape(B, S, D)
    out = out @ Wo + bo
    return (x * (1.0 - g) + out * g).astype(np.float32)


def kernel(x, gate, Wq, bq, Wk, bk, Wv, bv, Wo, bo, _profile=None):
    x = np.asarray(x, np.float32)
    gate = np.asarray(gate)
    args = dict(x=x, gate=gate, Wq=np.asarray(Wq, np.float32),
                bq=np.asarray(bq, np.float32), Wk=np.asarray(Wk, np.float32),
                bk=np.asarray(bk, np.float32), Wv=np.asarray(Wv, np.float32),
                bv=np.asarray(bv, np.float32), Wo=np.asarray(Wo, np.float32),
                bo=np.asarray(bo, np.float32))

    idxs = [np.nonzero(gate[b])[0] for b in range(B)]
    n_act = [len(i) for i in idxs]
    # the compaction trick needs zero q/k/v biases and at least one active
    # and one inactive token per batch; otherwise fall back to exact numpy
    if (any(np.abs(args[k]).max() > 0 for k in ("bq", "bk", "bv"))
            or min(n_act) == 0 or max(n_act) == S):
        return _reference_fallback(**args)

    SA = ((max(n_act) + 127) // 128) * 128
    npdt = _NPDT[MM_DTYPE]

    in_maps = []
    for b in range(B):
        xa = np.zeros((SA, D), np.float32)
        xa[:n_act[b]] = x[b, idxs[b]]
        xT = np.ascontiguousarray(xa.T).astype(npdt)
        for g in range(GROUPS):
            cs = slice(g * DC, (g + 1) * DC)
            # each 128-row block of Wo rotated by +1 to match the shifted
            # osbp row layout (Z occupies psum row 0 on device)
            wo_g = args["Wo"][cs, :]
            wo_r = np.concatenate(
                [np.roll(wo_g[p * 128:(p + 1) * 128], 1, axis=0)
                 for p in range(DC // 128)], axis=0)
            in_maps.append({
                "xT": xT,
                "wq": np.ascontiguousarray(args["Wq"][:, cs]).astype(npdt),
                "wk": np.ascontiguousarray(args["Wk"][:, cs]).astype(npdt),
                "wv": np.ascontiguousarray(args["Wv"][:, cs]).astype(npdt),
                "wo": np.ascontiguousarray(wo_r).astype(npdt),
            })

    nc = _get_nc(SA)
    kw = dict(_profile) if _profile else {}
    kw.pop("result", None)
    res = run_bass_kernel_spmd(nc, in_maps, core_ids=list(range(NCORES)), **kw)
    if _profile is not None:
        _profile["result"] = res

    out = x.copy()
    for b in range(B):
        Y = np.zeros((SA, D), np.float32)
        for g in range(GROUPS):
            Y += np.asarray(res.results[b * GROUPS + g]["y"]).astype(np.float32)
        out[b, idxs[b]] = Y[:n_act[b]] + args["bo"]
    return out


# revision 23
# speedup vs baseline: 1.2582x; 1.0112x over previous
"""Entropy-gated multi-head attention on 8 Trainium2 NeuronCores.

Sharding: core c = b*4 + g handles batch b (of 2) and head-group g (4 of the
16 heads).  Tokens with gate==0 pass x through untouched and contribute
exactly zero k/v (zero biases), so the device only processes the compacted
active tokens (~half), with the softmax denominator corrected by the count of
inactive tokens: each inactive key contributes exp(0)=1 to the softmax sum
(scores vs. zeroed k are exactly 0) and nothing to the numerator (v=0).

Device math per core (no max-subtraction; scores are O(5) so exp is safe):
  QT = Wq_g^T x^T, KT = Wk_g^T x^T           [256, SA]
  V  = x Wv_g                                 [SA, 256]
  per head h: PT = exp((KT_h^T QT_h)/8)       [SA_k, SA_q]
              OT' = [V_h | 1]^T PT            [65, SA_q] (row 64 = colsum = Z_act)
              r = 1/(Z_act + (S - SA))        recip on [2,qn] THEN broadcast
              osb_h = OT * r                  (scaled attention out, lhsT form)
  Y(q, :) = sum_h osb_h^T Wo_h                (pair-packed K=128 matmuls)
Host sums the 4 per-group partial Y per batch, adds bo, scatters into x.

v2 changes vs baseline (183976ns):
  - Y matmuls pair-packed K=128 (half the Y instruction cycles)
  - reciprocal_approx_fast on [2,qn] pre-broadcast (was reciprocal on
    [64,qn] post-broadcast: ~30us of DVE)
  - CADD folded into the same small pre-broadcast op
  - y output in bf16 (half the output DMA)
  - s-granular software pipeline: scores(s) / exp(s) / AV(s-1) with filler
    jobs drained every round to keep the PE continuously busy (max p-state)
  - PSUM: st bufs=2 (1 bank each), ot bufs=4, aux bufs=2 -> exactly 8 banks
"""

import math
from contextlib import ExitStack

import numpy as np
import ml_dtypes

import concourse.bass as bass
import concourse.mybir as mybir
from concourse import bacc
import concourse.tile as tile
from concourse.bass_utils import run_bass_kernel_spmd

B, S, D = 2, 2048, 1024
H, DH = 16, 64
NCORES = 8
GROUPS = NCORES // B          # head-groups per batch = 4
HC = H // GROUPS              # heads per core = 4
DC = HC * DH                  # head-group width = 256

MM_DTYPE = "bf16"

_DT = {"bf16": mybir.dt.bfloat16, "f32": mybir.dt.float32}
_NPDT = {"bf16": ml_dtypes.bfloat16, "f32": np.float32}

f32 = mybir.dt.float32
bf16 = mybir.dt.bfloat16


def _chunks(total, step):
    out = []
    o = 0
    while o < total:
        out.append((o, min(step, total - o)))
        o += step
    return out


def _build(SA: int, dtype_tag: str) -> bass.Bass:
    DT = _DT[dtype_tag]
    nkt = D // 128            # 8 contraction tiles for projections
    nst = SA // 128           # token (key) tiles
    qch = _chunks(SA, 512)    # q chunks
    dch = _chunks(D, 512)     # output-dim chunks
    CADD = float(S - SA)      # inactive keys not represented by padding

    nc = bacc.Bacc()
    xT_d = nc.dram_tensor("xT", [D, SA], DT, kind="ExternalInput")
    wq_d = nc.dram_tensor("wq", [D, DC], DT, kind="ExternalInput")
    wk_d = nc.dram_tensor("wk", [D, DC], DT, kind="ExternalInput")
    wv_d = nc.dram_tensor("wv", [D, DC], DT, kind="ExternalInput")
    wo_d = nc.dram_tensor("wo", [DC, D], DT, kind="ExternalInput")
    y_d = nc.dram_tensor("y", [SA, D], bf16, kind="ExternalOutput")

    with tile.TileContext(nc) as tc, ExitStack() as ctx:
        singles = ctx.enter_context(tc.tile_pool(name="singles", bufs=1))
        pt_pool = ctx.enter_context(tc.tile_pool(name="pt", bufs=3))
        osb_pool = ctx.enter_context(tc.tile_pool(name="osb", bufs=3))
        otmp_pool = ctx.enter_context(tc.tile_pool(name="otmp", bufs=2))
        zsb_pool = ctx.enter_context(tc.tile_pool(name="zsb", bufs=2))
        rbc_pool = ctx.enter_context(tc.tile_pool(name="rbc", bufs=3))
        yout_pool = ctx.enter_context(tc.tile_pool(name="yout", bufs=3))
        zscr_pool = ctx.enter_context(tc.tile_pool(name="zscr", bufs=2,
                                                   space="DRAM"))
        # PSUM budget: st(pair) 1x2 + ot 4x1 + aux 2x1 = 8 banks
        st_ps = ctx.enter_context(tc.tile_pool(name="stps", bufs=1, space="PSUM"))
        ot_ps_pool = ctx.enter_context(tc.tile_pool(name="otps", bufs=4, space="PSUM"))
        aux_ps = ctx.enter_context(tc.tile_pool(name="auxps", bufs=2, space="PSUM"))

        # ---- persistent SBUF; DMA order = consumption order for fast start
        wk_sb = singles.tile([128, nkt, DC], DT)
        wq_sb = singles.tile([128, nkt, DC], DT)
        wv_sb = singles.tile([128, nkt, DC], DT)
        xt = singles.tile([128, nkt, SA], DT)
        (c0, c0n) = qch[0]
        for t in range(nkt):
            nc.sync.dma_start(wk_sb[:, t, :], wk_d[t * 128:(t + 1) * 128, :])
            nc.sync.dma_start(xt[:, t, c0:c0 + c0n],
                              xT_d[t * 128:(t + 1) * 128, c0:c0 + c0n])
            nc.sync.dma_start(wq_sb[:, t, :], wq_d[t * 128:(t + 1) * 128, :])
        for t in range(nkt):
            nc.sync.dma_start(wv_sb[:, t, :], wv_d[t * 128:(t + 1) * 128, :])
        for (q0, qn) in qch[1:]:
            for t in range(nkt):
                nc.sync.dma_start(xt[:, t, q0:q0 + qn],
                                  xT_d[t * 128:(t + 1) * 128, q0:q0 + qn])
        wo_sb = []
        for p in range(HC // 2):
            w = singles.tile([128, D], DT, tag=f"wo{p}", name=f"wo{p}")
            nc.sync.dma_start(w, wo_d[p * 128:(p + 1) * 128, :])
            wo_sb.append(w)

        # ---- projections: QT/KT [256, SA], V (augmented with ones) ----
        qt = [singles.tile([128, SA], DT, tag=f"qt{m}", name=f"qt{m}")
              for m in range(2)]
        kt = [singles.tile([128, SA], DT, tag=f"kt{m}", name=f"kt{m}")
              for m in range(2)]
        v_aug = singles.tile([128, nst, HC, 65], DT)

        # ACT exp-table preload: tiny dummy exp as the very first ACT
        # instruction so the ~1.3us table load overlaps the input DMAs.
        dummy = singles.tile([1, 16], f32, tag="dummy", name="dummy")
        nc.vector.memset(dummy, 0.0)
        nc.scalar.activation(dummy, dummy,
                             mybir.ActivationFunctionType.Exp, scale=1.0)
        # ones column FIRST: the AV output then has Z on psum partition 0,
        # where the (partition-offset-buggy) fast reciprocal can run without
        # a partition-relocating DMA hop.  V lives in columns 1:65; the host
        # pre-rotates each 128-row block of Wo by +1 to match the shifted
        # row layout of osbp.
        nc.vector.memset(v_aug[:, :, :, 0:1], 1.0)

        # coverage tracking: 128-token tile indices of qt/kt already emitted,
        # so the round loop can assert its reads are behind the writes
        qk_cov = {(w, m): set() for w in ("q", "k") for m in range(2)}

        def proj_qk(kind, m, dst, w_sb, q0, qn):
            ps = aux_ps.tile([128, 512], f32, tag="aux", name="ps")
            for t in range(nkt):
                nc.tensor.matmul(
                    ps[:, :qn],
                    w_sb[:, t, m * 128:(m + 1) * 128],
                    xt[:, t, q0:q0 + qn],
                    start=(t == 0), stop=(t == nkt - 1))
            nc.vector.tensor_copy(dst[m][:, q0:q0 + qn], ps[:, :qn])
            qk_cov[(kind, m)].update(range(q0 // 128, (q0 + qn) // 128))

        v_done = set()

        def proj_v(s):
            v_done.add(s)
            ps = aux_ps.tile([128, 512], f32, tag="aux", name="ps")
            for t in range(nkt):
                nc.tensor.matmul(
                    ps[:, :DC],
                    xt[:, t, s * 128:(s + 1) * 128],
                    wv_sb[:, t, :],
                    start=(t == 0), stop=(t == nkt - 1))
            for h in range(HC):
                nc.vector.tensor_copy(v_aug[:, s, h, 1:65],
                                      ps[:, h * 64:(h + 1) * 64])

        # filler queue: independent ~850ns PE quanta drained between rounds
        jobs = []

        def drain(k):
            for _ in range(min(k, len(jobs))):
                jobs.pop(0)()

        def qk_halves(kind, m, q0, qn):
            # split a projection chunk into <=256-wide quanta
            dst, w_sb = (qt, wq_sb) if kind == "q" else (kt, wk_sb)
            out = []
            for (o, n) in _chunks(qn, 256):
                out.append(lambda a=q0 + o, b=n:
                           proj_qk(kind, m, dst, w_sb, a, b))
            return out

        # PE clock warmup: ~25 dummy matmuls keep the PE busy from t=0 while
        # the first input DMAs are in flight, so the HAM clock-gate is fully
        # open when real work starts.  dumA/dumB are never DMA'd (garbage).
        dumA = singles.tile([128, 128], DT, tag="dumA", name="dumA")
        dumB = singles.tile([128, 128], DT, tag="dumB", name="dumB")
        nc.vector.memset(dumA, 0.0)
        nc.vector.memset(dumB, 0.0)
        dum_ps = ot_ps_pool.tile([128, 512], f32, tag="ot", name="dum_ps")
        for _ in range(18):
            nc.tensor.matmul(dum_ps[:, :128], dumA, dumB,
                             start=True, stop=True)

        # warmup: KT(m0) for the FULL key range (chunk0's scores sweep all
        # key tiles) + QT(m0) chunk0 + first two V tiles; the rest queued in
        # an order that respects when the round loop consumes it.
        for (q0, qn) in qch:
            for j in qk_halves("k", 0, q0, qn):
                j()
        for j in qk_halves("q", 0, c0, c0n):
            j()
        proj_v(0)
        proj_v(1)
        vj = [lambda s=s: proj_v(s) for s in range(2, nst)]
        k1 = [j for (q0, qn) in qch for j in qk_halves("k", 1, q0, qn)]
        jobs += vj[:3]                       # V2-V4
        jobs += k1[:2]                       # KT m1 chunk0
        jobs += qk_halves("q", 1, c0, c0n)   # QT m1 chunk0 (pair1 round 0)
        jobs += vj[3:]                       # V5...
        jobs += k1[2:]                       # KT m1 rest (pair1 s>=4)
        for (q0, qn) in qch[1:]:
            jobs += qk_halves("q", 0, q0, qn)
            jobs += qk_halves("q", 1, q0, qn)

        # output projection: pair-packed K=128 accumulating matmuls;
        # one (jt, dch) quantum per job (~850ns of PE)
        def y_quantum(q0, qn, jt, d0, dn, osb_pair):
            qtn = min(128, qn - jt * 128)
            yp = aux_ps.tile([128, 512], f32, tag="aux", name="yp")
            for p in range(HC // 2):
                nc.tensor.matmul(
                    yp[:qtn, :dn],
                    osb_pair[p][:, jt * 128:jt * 128 + qtn],
                    wo_sb[p][:, d0:d0 + dn],
                    start=(p == 0), stop=(p == HC // 2 - 1))
            yo = yout_pool.tile([128, 512], bf16, tag="yo", name="yo")
            nc.vector.tensor_copy(yo[:qtn, :dn], yp[:qtn, :dn])
            nc.sync.dma_start(
                y_d[q0 + jt * 128: q0 + jt * 128 + qtn, d0:d0 + dn],
                yo[:qtn, :dn])

        # ---- attention: s-granular pipeline per (chunk, pair) ----
        for ci, (q0, qn) in enumerate(qch):
            osb_c = [None, None]
            for p in range(HC // 2):
                m = p
                h0, h1 = 2 * p, 2 * p + 1
                ot_ps = {h: ot_ps_pool.tile([65, 512], f32, tag="ot",
                                            name="ot_ps")
                         for h in (h0, h1)}
                # rounds process GSZ s-tiles each so narrow chunks still get
                # ~1024-element exp instructions and few semaphore rounds
                GSZ = max(1, 512 // qn)
                prev_grp = None   # (pt_tile, [s indices])
                for g0 in range(0, nst, GSZ):
                    grp = list(range(g0, min(g0 + GSZ, nst)))
                    # filler first: keeps PE busy while ACT runs prev exp
                    drain(2 if (ci == 0 and p == 0 and g0 < 6) else 1)
                    # both heads share one 2-bank st tile -> one exp instr
                    st = st_ps.tile([128, 2, GSZ, 512 // GSZ], f32,
                                    tag="st", name="st_ps")
                    assert all(t in qk_cov[("q", m)] for t in
                               range(q0 // 128, (q0 + qn) // 128)), \
                        f"QT m={m} chunk {q0} not emitted before scores"
                    for j, s in enumerate(grp):
                        assert s in qk_cov[("k", m)], \
                            f"KT m={m} tile {s} not emitted before scores"
                        for i, h in enumerate((h0, h1)):
                            r0 = (h % 2) * 64
                            nc.tensor.matmul(
                                st[:, i, j, :qn],
                                kt[m][r0:r0 + 64, s * 128:(s + 1) * 128],
                                qt[m][r0:r0 + 64, q0:q0 + qn],
                                start=True, stop=True,
                                tile_position=(r0, 0))
                    pt = pt_pool.tile([128, 2, GSZ, 512 // GSZ], DT,
                                      tag="pt", name="pt")
                    ng = len(grp)
                    nc.scalar.activation(
                        pt[:, :, :ng, :qn], st[:, :, :ng, :qn],
                        mybir.ActivationFunctionType.Exp, scale=0.125)
                    # AV lagged one group so prev exp is done when PE arrives
                    if prev_grp is not None:
                        ppt, pgrp = prev_grp
                        for j, sp in enumerate(pgrp):
                            assert sp in v_done, \
                                f"proj_v not emitted before OT use: {sp}"
                            for i, h in enumerate((h0, h1)):
                                nc.tensor.matmul(
                                    ot_ps[h][:, :qn],
                                    v_aug[:, sp, h, :],
                                    ppt[:, i, j, :qn],
                                    start=(sp == 0), stop=False)
                    prev_grp = (pt, grp)
                drain(1)
                ppt, pgrp = prev_grp
                for j, sp in enumerate(pgrp):
                    assert sp in v_done
                    for i, h in enumerate((h0, h1)):
                        nc.tensor.matmul(
                            ot_ps[h][:, :qn],
                            v_aug[:, sp, h, :],
                            ppt[:, i, j, :qn],
                            start=(sp == 0), stop=(sp == nst - 1))

                # Z path, per head (so h0's chain never waits on h1):
                # Z sits on psum partition 0 -> +CADD and fast reciprocal
                # run back-to-back on DVE at partition 0 (no relocation DMA),
                # then DRAM hop -> partition-broadcast -> scale.
                # osbp row layout (host pre-rotates Wo blocks to match):
                #   row 0      = h1 hd 127
                #   rows 1-64  = h0 hd 0-63
                #   rows 65-127= h1 hd 64-126
                osbp = osb_pool.tile([128, 512], DT, tag="osbp", name="osbp")
                osb_c[p] = osbp
                for i, h in enumerate((h0, h1)):
                    zq = zsb_pool.tile([1, 512], f32, tag="zq", name="zq")
                    nc.vector.tensor_scalar(
                        out=zq[0:1, :qn], in0=ot_ps[h][0:1, :qn],
                        scalar1=CADD, scalar2=None, op0=mybir.AluOpType.add)
                    nc.vector.reciprocal_approx_fast(zq[0:1, :qn],
                                                     zq[0:1, :qn])
                    zd = zscr_pool.tile([1, 512], f32, tag="zd", name="zd")
                    nc.sync.dma_start(zd[0:1, :qn], zq[0:1, :qn])
                    rb = rbc_pool.tile([65, 512], f32, tag="rb", name="rb")
                    nc.sync.dma_start(rb[0:65, :qn],
                                      zd[0:1, :qn].to_broadcast((65, qn)))
                    # STT covers rows 0-64 (start partition must be 0/32/64);
                    # row 0 is Z*r garbage, overwritten by the h1 relocate.
                    if i == 0:
                        nc.vector.scalar_tensor_tensor(
                            out=osbp[0:65, :qn], in0=ot_ps[h][0:65, :qn],
                            scalar=1.0, in1=rb[0:65, :qn],
                            op0=mybir.AluOpType.mult,
                            op1=mybir.AluOpType.mult)
                    else:
                        otmp = otmp_pool.tile([65, 512], DT, tag="otmp",
                                              name="otmp")
                        nc.vector.scalar_tensor_tensor(
                            out=otmp[0:65, :qn], in0=ot_ps[h][0:65, :qn],
                            scalar=1.0, in1=rb[0:65, :qn],
                            op0=mybir.AluOpType.mult,
                            op1=mybir.AluOpType.mult)
                        nc.sync.dma_start(osbp[65:128, :qn],
                                          otmp[1:64, :qn])
                        nc.sync.dma_start(osbp[0:1, :qn],
                                          otmp[64:65, :qn])

                # last chunk: start the Y accumulation for pair0 while
                # pair1's rounds still run, closing it right after pair1's
                # scale -- shortens the serial tail by one pair's Z chain
                is_last = ci == len(qch) - 1
                n_jt = (qn + 127) // 128
                split_ok = is_last and n_jt * len(dch) <= 2
                if split_ok and p == 0:
                    drain(len(jobs))
                    y_open = []
                    for jt in range(n_jt):
                        qtn = min(128, qn - jt * 128)
                        for (d0, dn) in dch:
                            yp = aux_ps.tile([128, 512], f32, tag="aux",
                                             name="yp")
                            nc.tensor.matmul(
                                yp[:qtn, :dn],
                                osb_c[0][:, jt * 128:jt * 128 + qtn],
                                wo_sb[0][:, d0:d0 + dn],
                                start=True, stop=False)
                            y_open.append((yp, jt, qtn, d0, dn))
                if split_ok and p == 1:
                    for (yp, jt, qtn, d0, dn) in y_open:
                        nc.tensor.matmul(
                            yp[:qtn, :dn],
                            osb_c[1][:, jt * 128:jt * 128 + qtn],
                            wo_sb[1][:, d0:d0 + dn],
                            start=False, stop=True)
                        yo = yout_pool.tile([128, 512], bf16, tag="yo",
                                            name="yo")
                        nc.vector.tensor_copy(yo[:qtn, :dn], yp[:qtn, :dn])
                        nc.sync.dma_start(
                            y_d[q0 + jt * 128: q0 + jt * 128 + qtn,
                                d0:d0 + dn],
                            yo[:qtn, :dn])

            if not split_ok:
                for jt in range((qn + 127) // 128):
                    for (d0, dn) in dch:
                        jobs.append(
                            lambda a=q0, b=qn, j=jt, e0=d0, en=dn,
                            o=tuple(osb_c):
                            y_quantum(a, b, j, e0, en, o))
        drain(len(jobs))
    nc.compile()
    return nc


_nc_cache: dict = {}


def _get_nc(SA: int):
    key = (SA, MM_DTYPE)
    if key not in _nc_cache:
        _nc_cache[key] = _build(SA, MM_DTYPE)
    return _nc_cache[key]


def _reference_fallback(x, gate, Wq, bq, Wk, bk, Wv, bv, Wo, bo):
    g = gate.astype(x.dtype)[..., None]
    q = (x @ Wq + bq) * g
    k = (x @ Wk + bk) * g
    v = (x @ Wv + bv) * g

    def split(t):
        return t.reshape(B, S, H, DH).transpose(0, 2, 1, 3)

    q, k, v = split(q), split(k), split(v)
    sc = np.einsum('bhqd,bhkd->bhqk', q, k) / np.float32(math.sqrt(DH))
    sc = sc - sc.max(axis=-1, keepdims=True)
    e = np.exp(sc)
    attn = e / e.sum(axis=-1, keepdims=True)
    out = np.einsum('bhqk,bhkd->bhqd', attn, v)
    out = out.transpose(0, 2, 1, 3).reshape(B, S, D)
    out = out @ Wo + bo
    return (x * (1.0 - g) + out * g).astype(np.float32)


def kernel(x, gate, Wq, bq, Wk, bk, Wv, bv, Wo, bo, _profile=None):
    x = np.asarray(x, np.float32)
    gate = np.asarray(gate)
    args = dict(x=x, gate=gate, Wq=np.asarray(Wq, np.float32),
                bq=np.asarray(bq, np.float32), Wk=np.asarray(Wk, np.float32),
                bk=np.asarray(bk, np.float32), Wv=np.asarray(Wv, np.float32),
                bv=np.asarray(bv, np.float32), Wo=np.asarray(Wo, np.float32),
                bo=np.asarray(bo, np.float32))

    idxs = [np.nonzero(gate[b])[0] for b in range(B)]
    n_act = [len(i) for i in idxs]
    # the compaction trick needs zero q/k/v biases and at least one active
    # and one inactive token per batch; otherwise fall back to exact numpy
    if (any(np.abs(args[k]).max() > 0 for k in ("bq", "bk", "bv"))
            or min(n_act) == 0 or max(n_act) == S):
        return _reference_fallback(**args)

    SA = ((max(n_act) + 127) // 128) * 128
    npdt = _NPDT[MM_DTYPE]

    in_maps = []
    for b in range(B):
        xa = np.zeros((SA, D), np.float32)
        xa[:n_act[b]] = x[b, idxs[b]]
        xT = np.ascontiguousarray(xa.T).astype(npdt)
        for g in range(GROUPS):
            cs = slice(g * DC, (g + 1) * DC)
            # each 128-row block of Wo rotated by +1 to match the shifted
            # osbp row layout (Z occupies psum row 0 on device)
            wo_g = args["Wo"][cs, :]
            wo_r = np.concatenate(
                [np.roll(wo_g[p * 128:(p + 1) * 128], 1, axis=0)
                 for p in range(DC // 128)], axis=0)
            in_maps.append({
                "xT": xT,
                "wq": np.ascontiguousarray(args["Wq"][:, cs]).astype(npdt),
                "wk": np.ascontiguousarray(args["Wk"][:, cs]).astype(npdt),
                "wv": np.ascontiguousarray(args["Wv"][:, cs]).astype(npdt),
                "wo": np.ascontiguousarray(wo_r).astype(npdt),
            })

    nc = _get_nc(SA)
    kw = dict(_profile) if _profile else {}
    kw.pop("result", None)
    res = run_bass_kernel_spmd(nc, in_maps, core_ids=list(range(NCORES)), **kw)
    if _profile is not None:
        _profile["result"] = res

    out = x.copy()
    for b in range(B):
        Y = np.zeros((SA, D), np.float32)
        for g in range(GROUPS):
            Y += np.asarray(res.results[b * GROUPS + g]["y"]).astype(np.float32)
        out[b, idxs[b]] = Y[:n_act[b]] + args["bo"]
    return out
